# revision 67
# baseline (speedup 1.0000x reference)
"""Trainium2 Bass kernel for nn_Attention_30468497997979.

Reference computation (per batch b of 8):
    X = hidden_states[b,:,0,:]              # (C=768, S=384)
    Q/K/V = W @ X + b                       # 1x1 conv == channel matmul
    per head h (12 heads, head dim 64, channel c = d*12 + h):
        scores = (Q_h^T K_h) / 8, mask (keys k < q masked), softmax over k
        attn_h = V_h @ softmax
    out = Wo @ concat_heads(attn)           # channel c = h*64 + d

Sharding: pure data-parallel, one batch per NeuronCore (8 cores).

Per-core kernel design (v2 — rebalanced engines, consolidated DMAs):
  - Host pre-permutes W_{q,k,v} rows to head-major channel order
    (c' = h*64 + d) and transposes all weights to [c_in, c_out].
    1/sqrt(d) folded into Wq/bq; V bias folded through attention into an
    output bias Wo @ bv (softmax rows sum to 1). All matmul data bf16.
  - Every dma_start costs ~0.7us of issuing-engine time regardless of
    size, so inputs load as ONE monolithic DMA per tensor ([128, 6, *]
    rearranged), with wq split in two so Q-proj of chunk 0 starts early.
    Small constants (biases, 0/1 triangle mask) ride in one packed
    block. All ones-constants are memset on-chip.
  - scores are computed transposed ([k, q], keys on partitions):
    lhsT = K_h k-chunk, rhs = Q_h. Causal trimming: k-chunk kc only
    needs q-columns 0..(kc+1)*128. Per head two PSUM tiles:
    A = [kc0 q0:128 | kc1 q0:256] (one bank), B = [kc2 q0:384].
  - softmax needs no max-subtraction (scores are O(1)). The mask is
    applied multiplicatively AFTER exp: exp(s+m) = exp(s)*{0,1}, so the
    Scalar engine exps straight out of PSUM (2 calls/head) and the
    GpSimd engine (no PSUM port) multiplies the three diagonal
    [128,128] sub-blocks by a 0/1 triangle in bf16 SBUF.
  - attn@V contracts over k on partitions (lhsT = per-head V^T tile
    with a fused ones-column computing the softmax denominator as PSUM
    row 64). DVE copies rows 0:64 to an unnormalized-attn tile (head
    parity picks partition half), Scalar copies the denominator row
    into a staging row.
  - Normalization: batched DVE reciprocal over [3n,128]-shaped sums
    (reciprocal cost is free-dim bound), one DMA repack per group, then
    per-CHUNK (2 heads at once) a K=2 matmul broadcasts both heads'
    1/sum rows across the right partition halves (lhsT is a 0/1
    selector), and one DVE multiply normalizes the whole [128,384]
    chunk. Groups (heads 0-7, 8-11) keep the tail short.
  - Engine budget: PE ~34us (bound), Scalar = exps+sums+output bias,
    DVE = bias-adds/copies/reciprocal/normalize, GpSimd = masks+DMAs.
"""

import numpy as np

B, C, S, H, D = 8, 768, 384, 12, 64
NC_CHUNKS = C // 128  # 6

_STATE = {}


# --------------------------------------------------------------------------
# Workaround: this walrus build rejects the multi-wait InstDrain that
# TileContext emits at exit ("Too many sync wait commands"). Split the
# drain's sem waits onto standalone sync-engine wait instructions.
def _patch_walrus_ldw_opt():
    """Enable walrus's load-weight pipelining (ldw-opt): overlaps each
    matmul's LDWEIGHTS with the previous matmul's execution."""
    import os
    import concourse.bass_utils as bu

    if os.environ.get("KERNEL_LDW_OPT") != "1":
        return
    if getattr(bu, "_ldw_opt_patch", False):
        return
    orig = bu.run_command

    def patched(argv, **kwargs):
        argv = [
            a.replace("--enable-ldw-opt=false", "--enable-ldw-opt=true")
            if isinstance(a, str)
            else a
            for a in argv
        ]
        return orig(argv, **kwargs)

    bu.run_command = patched
    bu._ldw_opt_patch = True


def _patch_tile_drain():
    import concourse.tile as tile_mod
    from concourse.vector_clock import ScopedClock
    from bass_rust import SyncInfo

    if getattr(tile_mod.TileContext, "_drain_split_patch", False):
        return

    def _drain_and_barrier_split(self, tick_clock, wait_clock):
        nc = self.nc
        assert self.sems is not None
        handles = {}
        for h in self.sems.allocated().values():
            handles[h.num] = h
            handles[h.name] = h

        probe = nc.sync.nop()
        wait_clock.add_sem_waits(
            probe.ins, ScopedClock({None: tick_clock.global_clock})
        )
        waits = list(probe.ins.sync_info.on_wait)
        probe.ins.sync_info = SyncInfo(on_wait=[], on_update=[])
        for w in waits:
            h = handles.get(w.id) or handles.get(w.ant_name)
            if h is not None:
                nc.sync.wait_ge(h, w.wait_value)
            else:
                n2 = nc.sync.nop()
                n2.ins.sync_info = SyncInfo(on_wait=[w], on_update=[])

        drain_inst = nc.sync.drain()
        wait_clock.add_sem_waits(
            drain_inst.ins, ScopedClock({None: tick_clock.global_clock})
        )
        if list(drain_inst.ins.sync_info.on_wait):
            drain_inst.ins.sync_info = SyncInfo(on_wait=[], on_update=[])

        nc.all_engine_barrier()
        popped = nc._tile_sem_poison_stack.pop()
        assert popped is self._sem_poison
        nc.clear_and_free_semaphores(list(self.sems.allocated().values()))
        nc.all_engine_barrier()

        # This walrus codegen supports at most ONE sem wait per
        # instruction. Move extra waits onto same-engine nop carriers
        # inserted just before the instruction (engine queues execute in
        # order, so the semantics are identical).
        import concourse.mybir as mybir

        k = 0
        for f in nc.m.functions:
            for bb in f.blocks:
                new_insts = []
                for inst in bb.instructions:
                    si = inst.sync_info
                    waits = list(si.on_wait) if si else []
                    if len(waits) > 1:
                        for w in waits[:-1]:
                            nop = mybir.InstNoOp(name=f"I-wsplit-{k}")
                            k += 1
                            nop.engine = inst.engine
                            nop.sync_info = SyncInfo(on_wait=[w], on_update=[])
                            nc.register_instruction(nop)
                            new_insts.append(nop)
                        inst.sync_info = SyncInfo(
                            on_wait=[waits[-1]], on_update=list(si.on_update)
                        )
                    new_insts.append(inst)
                bb.instructions = new_insts

    tile_mod.TileContext._drain_and_barrier = _drain_and_barrier_split
    tile_mod.TileContext._drain_split_patch = True


# --------------------------------------------------------------------------
def _build_nc():
    import concourse.bass as bass
    import concourse.mybir as mybir
    import concourse.tile as tile

    _patch_tile_drain()
    _patch_walrus_ldw_opt()

    f32 = mybir.dt.float32
    f32r = mybir.dt.float32r
    bf16 = mybir.dt.bfloat16
    Ident = mybir.ActivationFunctionType.Identity
    Copy = mybir.ActivationFunctionType.Copy
    Exp = mybir.ActivationFunctionType.Exp

    nc = bass.Bass()
    # All tensors arrive host-packed in their exact SBUF layouts so every
    # DMA moves one contiguous multi-KB run per partition (DMA throughput
    # is descriptor-rate bound; descriptors cover one partition row each).
    # t1 = [x | wq chunk0 | wk chunk0] — one DMA gates the first
    # projections (in-queue completion order across separate DMAs is
    # unreliable). t2 = remaining wq/wk slices grouped per out-chunk.
    t1_d = nc.dram_tensor("t1", [128, 3840], bf16, kind="ExternalInput")
    t2_d = [
        nc.dram_tensor(f"t2{i}", [128, 1, 2, NC_CHUNKS, 128], bf16, kind="ExternalInput")
        for i in range(5)
    ]
    wv_d = nc.dram_tensor("wvt", [128, 2, NC_CHUNKS, 384], bf16, kind="ExternalInput")
    wo_d = nc.dram_tensor("wot", [128, NC_CHUNKS, C], bf16, kind="ExternalInput")
    # packed constants [128, 146] f32:
    #   cols 0:6 bq (col=chunk), 6:12 bk, 12:18 obias (= Wo @ bv'),
    #   cols 18:146 = [128, 256] bf16 = 0/1 lower-triangle (k>=q) twice
    cst_d = nc.dram_tensor("cst", [128, 146], f32, kind="ExternalInput")
    sel_d = nc.dram_tensor("sel", [2, 128], f32, kind="ExternalInput")
    y_d = nc.dram_tensor("y", [128, NC_CHUNKS, S], bf16, kind="ExternalOutput")

    with tile.TileContext(nc) as tc:
        with (
            tc.tile_pool(name="persist", bufs=1) as persist,
            tc.tile_pool(name="epool", bufs=9) as epool,
            tc.tile_pool(name="psA", bufs=2, space="PSUM") as psA,
            tc.tile_pool(name="psS", bufs=4, space="PSUM") as psS,
            tc.tile_pool(name="psV", bufs=2, space="PSUM") as psV,
        ):
            # ---- input loads ------------------------------------------
            # DMA throughput is descriptor-rate bound (one descriptor per
            # partition row), so every tensor arrives host-packed in its
            # exact SBUF layout: each DMA moves one contiguous 1.5-9KB run
            # per partition. Only the DMAs the first projection needs are
            # emitted before it: the tile framework lowers multi-dep waits
            # to one conservative per-queue semaphore value, so any DMA
            # emitted earlier on the same queue would false-serialize it.
            t1 = persist.tile([128, 3840], bf16, tag="t1", name="t1")
            t2 = persist.tile(
                [128, 5, 2, NC_CHUNKS, 128], bf16, tag="t2", name="t2"
            )
            wv_sb = persist.tile([128, 2, NC_CHUNKS, 384], bf16, tag="wv", name="wv")
            wo_sb = persist.tile([128, NC_CHUNKS, C], bf16, tag="wo", name="wo")
            cst = persist.tile([128, 146], f32, tag="cst", name="cst")

            xt = t1[:, 0:2304].rearrange("p (cc s) -> p cc s", s=S)
            wqk0 = t1[:, 2304:3840].rearrange("p (w cc c) -> p w cc c", w=2, c=128)

            def wslice(w, oc, cc):
                # w: 0 = wq, 1 = wk; chunk 0 lives in t1, the rest in t2
                if oc == 0:
                    return wqk0[:, w, cc, :]
                return t2[:, oc - 1, w, cc, :]

            # DMA rings on one queue run CONCURRENTLY (not FIFO), so a
            # later dma_start steals bandwidth from an earlier one. Only
            # t1 + tiny constants issue up front; each later load is
            # released from an engine-queue position that already depends
            # on prior data (staged below in the schedule).
            nc.scalar.dma_start(t1[:], t1_d[:, :])
            nc.gpsimd.dma_start(cst[:], cst_d[:, :])

            def stage_load(stage):
                if stage < 2:
                    # wq/wk chunks 1-2 right behind t1 on the scalar queue
                    nc.scalar.dma_start(
                        t2[:, stage : stage + 1], t2_d[stage][:, :, :, :, :]
                    )
                elif stage == 2:
                    nc.scalar.dma_start(wv_sb[:], wv_d[:, :, :, :])
                elif stage == 3:
                    for i in (2, 3, 4):
                        nc.gpsimd.dma_start(
                            t2[:, i : i + 1], t2_d[i][:, :, :, :, :]
                        )
                elif stage == 4:
                    nc.gpsimd.dma_start(wo_sb[:], wo_d[:, :, :])

            # [128, 2, 128] view of the doubled 0/1 triangle
            tri2 = cst[:, 18:146].bitcast(bf16).rearrange("p (a q) -> p a q", q=128)

            # ---- on-chip constants -----------------------------------
            # vt[sq][k_local, h, 0:64] = V'[c', s]^T ; col 64 = 1.0 (fused
            # softmax-denominator column). sel2 = 0/1 selector for the K=2
            # normalize broadcast (row p lights up partition half p).
            vt = []
            for sq in range(3):
                t = persist.tile([128, H, D + 1], bf16, tag=f"vt{sq}", name=f"vt{sq}")
                nc.gpsimd.memset(t[:, :, D : D + 1], 1.0)
                vt.append(t)
            sel2 = persist.tile([2, 128], f32r, tag="sel2", name="sel2")
            nc.gpsimd.dma_start(sel2[:], sel_d[:, :].bitcast(f32r))

            # ---- persistent working tiles ----------------------------
            q_sb = [
                persist.tile([128, S], bf16, tag=f"q{oc}", name=f"q{oc}")
                for oc in range(NC_CHUNKS)
            ]
            k_sb = [
                persist.tile([128, S], bf16, tag=f"k{oc}", name=f"k{oc}")
                for oc in range(NC_CHUNKS)
            ]
            attn_sb = [
                persist.tile([128, S], bf16, tag=f"at{oc}", name=f"at{oc}")
                for oc in range(NC_CHUNKS)
            ]
            # unnormalized attn [rows 0:64] with the fused softmax
            # denominator as row 64, one column block per head (ordered
            # parity-major within each normalize group so the sums-gather
            # DMA reads one contiguous strip of partition 64)
            AU_g = [
                persist.tile([D + 1, 8, S], f32, tag="au0", name="au0"),
                persist.tile([D + 1, 4, S], f32, tag="au1", name="au1"),
            ]
            sums_g = [
                persist.tile([24, 128], f32, tag="sm0", name="sm0"),
                persist.tile([12, 128], f32, tag="sm1", name="sm1"),
            ]
            rinv_g = [
                persist.tile([24, 128], f32, tag="ri0", name="ri0"),
                persist.tile([12, 128], f32, tag="ri1", name="ri1"),
            ]
            rr_g = [
                persist.tile([2, 4, S], f32r, tag="rr0", name="rr0"),
                persist.tile([2, 2, S], f32r, tag="rr1", name="rr1"),
            ]
            ot = persist.tile([128, NC_CHUNKS, S], bf16, tag="ot", name="ot")

            # ---- stage helpers ---------------------------------------
            def qkproj(oc, w, bcol, out_sb):
                ps = psA.tile([128, S], f32, tag="proj", name="proj")
                for cc in range(NC_CHUNKS):
                    nc.tensor.matmul(
                        ps[:],
                        wslice(w, oc, cc),
                        xt[:, cc, :],
                        start=(cc == 0),
                        stop=(cc == NC_CHUNKS - 1),
                    )
                nc.vector.tensor_scalar_add(out_sb[:], ps[:], cst[:, bcol : bcol + 1])

            def vproj(sq, half):
                ps = psA.tile([128, S], f32, tag="proj", name="proj")
                for cc in range(NC_CHUNKS):
                    nc.tensor.matmul(
                        ps[:],
                        xt[:, cc, sq * 128 : (sq + 1) * 128],
                        wv_sb[:, half, cc, :],
                        start=(cc == 0),
                        stop=(cc == NC_CHUNKS - 1),
                    )
                dst = vt[sq][:, half * 6 : (half + 1) * 6, 0:D]
                src = ps[:].rearrange("p (h d) -> p h d", d=D)
                if half == 0:
                    nc.vector.tensor_copy(dst, src)
                else:
                    nc.scalar.activation(dst, src, Copy)

            def scores(h):
                # psum tile A: [kc0 | kc1] (cols 0:128 = q0:128 over keys
                # 0:128; cols 128:384 = q0:256 over keys 128:256), tile B:
                # kc2 q0:384. exp straight from PSUM; 0/1 triangle applied
                # after on the three diagonal sub-blocks (gpsimd, SBUF).
                oc, prow = h // 2, (h % 2) * D
                Qh = q_sb[oc][prow : prow + D, :]
                Kh = k_sb[oc][prow : prow + D, :]
                psa = psS.tile([128, S], f32, tag="sc", name="sc")
                nc.tensor.matmul(
                    psa[:, 0:128], Kh[:, 0:128], Qh[:, 0:128],
                    start=True, stop=True, skip_group_check=True,
                )
                nc.tensor.matmul(
                    psa[:, 128:384], Kh[:, 128:256], Qh[:, 0:256],
                    start=True, stop=True, skip_group_check=True,
                )
                psb = psS.tile([128, S], f32, tag="sc", name="sc")
                nc.tensor.matmul(
                    psb[:], Kh[:, 256:384], Qh[:, 0:384], start=True, stop=True,
                )
                # eA is 512 wide so its two diagonal sub-blocks (cols 0:128
                # and 256:384) form one uniform-stride [128,2,128] AP for a
                # single masked multiply. Late heads mask on DVE (fast bf16
                # SBUF path) to shorten the tail chase; earlier heads on the
                # otherwise-idle GpSimd.
                eA = epool.tile([128, 512], bf16, tag="eA", name="eA")
                nc.scalar.activation(eA[:, 0:S], psa[:], Exp)
                eB = epool.tile([128, S], bf16, tag="eB", name="eB")
                nc.scalar.activation(eB[:], psb[:], Exp)
                eng = nc.vector if h >= 10 else nc.gpsimd
                diag2 = eA[:].rearrange("p (a q) -> p a q", q=256)[:, :, 0:128]
                eng.tensor_mul(diag2, diag2, tri2)
                eng.tensor_mul(eB[:, 256:384], eB[:, 256:384], tri2[:, 0, :])
                return eA, eB

            def av(h, eA, eB):
                # accumulate widest first so every element's first write
                # carries the start flag
                ps_av = psV.tile([D + 1, S], f32, tag="av", name="av")
                nc.tensor.matmul(
                    ps_av[:, 0:384], vt[2][:, h, :], eB[:, 0:384],
                    start=True, stop=False, skip_group_check=True,
                )
                nc.tensor.matmul(
                    ps_av[:, 0:256], vt[1][:, h, :], eA[:, 128:384],
                    start=False, stop=False, skip_group_check=True,
                )
                nc.tensor.matmul(
                    ps_av[:, 0:128], vt[0][:, h, :], eA[:, 0:128],
                    start=False, stop=True, skip_group_check=True,
                )
                g = 0 if h < 8 else 1
                su_idx = (h % 2) * 4 + h // 2 if h < 8 else (h % 2) * 2 + (h - 8) // 2
                dst = AU_g[g][0 : D + 1, su_idx, :]
                if h >= 4 and h % 2 == 1:
                    # split the end-phase copies across Scalar and DVE
                    nc.scalar.activation(dst, ps_av[:, :], Copy)
                else:
                    nc.vector.tensor_copy(dst, ps_av[:, :])

            def norm_sums_dma(g, oc0, n_oc):
                # sums -> [3n,128] parity-major rows (reciprocal is
                # free-dim bound, so spread over partitions)
                n_h = 2 * n_oc
                nc.sync.dma_start(sums_g[g][:], AU_g[g][D : D + 1, 0:n_h, :])

            def norm_recip(g):
                # reciprocal, then one repack DMA back to two partitions
                # (row = head parity)
                nc.vector.reciprocal(rinv_g[g][:], sums_g[g][:])
                nc.sync.dma_start(rr_g[g][:], rinv_g[g][:].bitcast(f32r))

            def norm_apply(g, oc0, j):
                # per chunk: one K=2 broadcast matmul lights each partition
                # half with its head's 1/sum row, two half-chunk multiplies
                oc = oc0 + j
                ps_r = psS.tile([128, S], f32, tag="sc", name="sc")
                nc.tensor.matmul(
                    ps_r[:], sel2[:], rr_g[g][:, j, :], start=True, stop=True,
                )
                for par in range(2):
                    h = 2 * oc + par
                    su_idx = (
                        (h % 2) * 4 + h // 2 if h < 8 else (h % 2) * 2 + (h - 8) // 2
                    )
                    nc.vector.tensor_mul(
                        attn_sb[oc][par * D : (par + 1) * D, :],
                        AU_g[g][0:D, su_idx, :],
                        ps_r[par * D : (par + 1) * D, :],
                    )

            o_ps = {}

            def oproj(oc, ccs, start, stop):
                if oc in o_ps:
                    ps = o_ps[oc]
                else:
                    ps = o_ps[oc] = psA.tile([128, S], f32, tag="proj", name="proj")
                for i, cc in enumerate(ccs):
                    nc.tensor.matmul(
                        ps[:],
                        wo_sb[:, cc, oc * 128 : (oc + 1) * 128],
                        attn_sb[cc],
                        start=(start and i == 0),
                        stop=(stop and i == len(ccs) - 1),
                        skip_group_check=True,
                    )
                if stop:
                    del o_ps[oc]
                    nc.scalar.activation(
                        ot[:, oc, :], ps[:], Ident, bias=cst[:, 12 + oc : 13 + oc]
                    )
                    if oc % 2 == 1:
                        nc.sync.dma_start(
                            y_d[:, oc - 1 : oc + 1, :], ot[:, oc - 1 : oc + 1, :]
                        )

            # ---- schedule --------------------------------------------
            # oc0-2 projections+scores run while wv loads; V-proj next;
            # then attn@V of chunk oc-3 leads each chunk's projections so
            # the Scalar exp latency hides behind independent PE work.
            # Normalization is grouped (heads 0-7, 8-11); o_proj chains
            # for oc0/oc1 open early to fill the group-1 reciprocal stall.
            e_tiles = {}
            for oc in (0, 1, 2, 3):
                qkproj(oc, 0, oc, q_sb[oc])
                qkproj(oc, 1, 6 + oc, k_sb[oc])
                for h in (2 * oc, 2 * oc + 1):
                    e_tiles[h] = scores(h)
                    if oc == 0:
                        stage_load(h)  # stages 0, 1
                if oc == 1:
                    stage_load(2)  # wv
                    stage_load(3)  # wq/wk chunks 3-5
                if oc == 2:
                    stage_load(4)  # wo
            for sq in range(3):
                for half in range(2):
                    vproj(sq, half)
            for oc in (4, 5):
                for h in (2 * oc - 8, 2 * oc - 7):
                    av(h, *e_tiles.pop(h))
                qkproj(oc, 0, oc, q_sb[oc])
                qkproj(oc, 1, 6 + oc, k_sb[oc])
                for h in (2 * oc, 2 * oc + 1):
                    e_tiles[h] = scores(h)
            for h in (4, 5, 6, 7):
                av(h, *e_tiles.pop(h))
            norm_sums_dma(0, 0, 4)
            av(8, *e_tiles.pop(8))
            av(9, *e_tiles.pop(9))
            av(10, *e_tiles.pop(10))
            av(11, *e_tiles.pop(11))
            norm_sums_dma(1, 4, 2)
            norm_recip(0)
            for j in range(4):
                norm_apply(0, 0, j)
            oproj(0, (0, 1, 2, 3), start=True, stop=False)
            oproj(1, (0, 1, 2, 3), start=True, stop=False)
            norm_recip(1)
            norm_apply(1, 4, 0)
            norm_apply(1, 4, 1)
            oproj(0, (4, 5), start=False, stop=True)
            oproj(1, (4, 5), start=False, stop=True)
            for oc in (2, 3, 4, 5):
                oproj(oc, (0, 1, 2, 3, 4, 5), start=True, stop=True)

    return nc


def _get_nc():
    if "nc" not in _STATE:
        _STATE["nc"] = _build_nc()
    return _STATE["nc"]


# --------------------------------------------------------------------------
def _prep_maps(inputs):
    import ml_dtypes

    bf16 = ml_dtypes.bfloat16

    hs = np.asarray(inputs["hidden_states"], dtype=np.float32)
    Wq = np.asarray(inputs["Wq"], dtype=np.float32)
    bq = np.asarray(inputs["bq"], dtype=np.float32)
    Wk = np.asarray(inputs["Wk"], dtype=np.float32)
    bk = np.asarray(inputs["bk"], dtype=np.float32)
    Wv = np.asarray(inputs["Wv"], dtype=np.float32)
    bv = np.asarray(inputs["bv"], dtype=np.float32)
    Wo = np.asarray(inputs["Wo"], dtype=np.float32)

    # head-major channel permutation: c' = h*64 + d  <-  c = d*12 + h
    idx = (np.arange(H)[:, None] + np.arange(D)[None, :] * H).reshape(C)
    scale = float(D) ** -0.5

    wqt = np.ascontiguousarray((scale * Wq[idx, :]).T).astype(bf16)
    wkt = np.ascontiguousarray(Wk[idx, :].T).astype(bf16)
    wvt = np.ascontiguousarray(Wv[idx, :].T).astype(bf16)
    wot = np.ascontiguousarray(Wo.T).astype(bf16)

    # packed constants [128, 146] f32
    cstf = np.zeros((128, 146), dtype=np.float32)
    cstf[:, 0:6] = (scale * bq[idx]).reshape(NC_CHUNKS, 128).T
    cstf[:, 6:12] = bk[idx].reshape(NC_CHUNKS, 128).T
    # V-bias folded through attention (softmax rows sum to 1):
    # out += Wo @ bv'
    cstf[:, 12:18] = (Wo @ bv[idx]).reshape(NC_CHUNKS, 128).T
    # 0/1 triangle: allowed keys are k >= q -> tri[k, q] = 1 iff k >= q
    tri = (
        np.tril(np.ones((128, 128), dtype=np.float32))
        .astype(bf16)
    )
    cstf[:, 18:146] = np.tile(tri, (1, 2)).view(np.float32)

    sel = np.zeros((2, 128), dtype=np.float32)
    sel[0, 0:64] = 1.0
    sel[1, 64:128] = 1.0

    # pack [c_in, c_out] weights into their SBUF layouts (see _build_nc)
    nch = NC_CHUNKS
    wqp = np.ascontiguousarray(
        wqt.reshape(nch, 128, nch, 128).transpose(1, 2, 0, 3)
    )  # [p, out_chunk, in_chunk, col]
    wkp = np.ascontiguousarray(wkt.reshape(nch, 128, nch, 128).transpose(1, 2, 0, 3))
    wvp = np.ascontiguousarray(
        wvt.reshape(nch, 128, 2, 384).transpose(1, 2, 0, 3)
    )  # [p, half, in_chunk, col]
    wop = np.ascontiguousarray(wot.reshape(nch, 128, C).transpose(1, 0, 2))

    t2 = np.stack([wqp[:, 1:6], wkp[:, 1:6]], axis=2)  # [128, 5, 2, 6, 128]
    w0 = np.concatenate(
        [wqp[:, 0].reshape(128, 768), wkp[:, 0].reshape(128, 768)], axis=1
    )
    shared = {"wvt": wvp, "wot": wop, "cst": cstf, "sel": sel}
    for i in range(5):
        shared[f"t2{i}"] = np.ascontiguousarray(t2[:, i : i + 1])
    maps = []
    for b in range(B):
        xb = hs[b, :, 0, :].astype(bf16)
        xp = xb.reshape(nch, 128, S).transpose(1, 0, 2).reshape(128, nch * S)
        t1 = np.ascontiguousarray(np.concatenate([xp, w0], axis=1))
        maps.append({"t1": t1, **shared})
    return maps


def _run(inputs, trace=False, **kwargs):
    from concourse.bass_utils import run_bass_kernel_spmd

    nc = _get_nc()
    in_maps = _prep_maps(inputs)
    res = run_bass_kernel_spmd(
        nc, in_maps, core_ids=list(range(B)), trace=trace, **kwargs
    )
    out = np.stack(
        [
            np.asarray(res.results[b]["y"])
            .astype(np.float32)
            .transpose(1, 0, 2)  # [p, cc, s] -> [cc, p, s]
            .reshape(C, S)
            for b in range(B)
        ],
        axis=0,
    )
    return out.reshape(B, C, 1, S), res


def kernel(**inputs):
    out, _ = _run(inputs, trace=False)
    return out


# revision 68
# speedup vs baseline: 1.1992x; 1.1992x over previous
"""Trainium2 Bass kernel for nn_Attention_30468497997979.

Reference computation (per batch b of 8):
    X = hidden_states[b,:,0,:]              # (C=768, S=384)
    Q/K/V = W @ X + b                       # 1x1 conv == channel matmul
    per head h (12 heads, head dim 64, channel c = d*12 + h):
        scores = (Q_h^T K_h) / 8, mask (keys k < q masked), softmax over k
        attn_h = V_h @ softmax
    out = Wo @ concat_heads(attn)           # channel c = h*64 + d

Sharding: pure data-parallel, one batch per NeuronCore (8 cores).

Per-core kernel design (v3):
  - Host pre-permutes W_{q,k,v} rows to head-major channel order
    (c' = h*64 + d), transposes all weights to [c_in, c_out], and PACKS
    every input into its exact SBUF layout ([128 partitions, ...]), so
    each DMA moves one contiguous multi-KB run per partition (DMA
    throughput is descriptor-rate bound, one descriptor per partition
    row; narrow column-sliced loads collapse to 256B packets and
    strangle the queues). 1/sqrt(d) folded into Wq/bq; V bias folded
    through attention into an output bias Wo @ bv (softmax rows sum to
    1). All matmul data bf16 (PSUM accumulation fp32).
  - DMA scheduling quirks this kernel works around: (1) each dma_start
    costs ~0.7us of issuing-engine time regardless of size; (2) rings
    on one queue drain CONCURRENTLY, so a later dma_start steals
    bandwidth from an earlier one — the t1 block [x | wq0 | wk0] that
    gates the first projection is issued alone, and every later load is
    released from an engine-queue position that already depends on
    prior data ("staged release"); (3) HBM is shared by all 8 cores, so
    load timing has several-us run-to-run variance.
  - scores are computed transposed ([k, q], keys on partitions):
    lhsT = K_h k-chunk, rhs = Q_h. Causal trimming: k-chunk kc only
    needs q-columns 0..(kc+1)*128. Per head two PSUM tiles:
    A = [kc0 q0:128 | kc1 q0:256] (one bank), B = [kc2 q0:384].
  - softmax needs no max-subtraction (scores are O(1)). The mask is
    applied multiplicatively AFTER exp: exp(s+m) = exp(s)*{0,1}, so the
    Scalar engine exps straight out of PSUM (2 calls/head) and the
    GpSimd engine (no PSUM port) multiplies the diagonal [128,128]
    sub-blocks by a 0/1 triangle in bf16 SBUF (DVE for the last heads
    to shorten the tail chase).
  - attn@V contracts over k on partitions (lhsT = per-head V^T tile
    with a fused ones-column computing the softmax denominator as PSUM
    row 64). One 65-row copy per head lands rows+denominator in a
    group's AU tile (parity-major column order). The two normalize
    groups (heads 0-7, 8-11) use SEPARATE AU tiles — a shared tile
    makes the group-0 sums-gather DMA a false WAR barrier against
    group-1 copies (coarse dependency tracking), which cost ~10us.
  - Normalization: batched DVE reciprocal over [3n,128]-shaped sums
    (reciprocal cost is free-dim bound), one DMA repack per group, then
    per-CHUNK (2 heads at once) a K=2 matmul broadcasts both heads'
    1/sum rows across the right partition halves (lhsT is a 0/1
    selector), and two half-chunk DVE multiplies normalize the chunk.
  - o_proj chains for oc0/oc1 open early (cc 0-3) to fill the group-1
    reciprocal stall; output stores as three 2-chunk packed DMAs.
  - Engine budget: PE ~33us of columns (bound), Scalar = exps + half
    the attn copies + output bias, DVE = proj bias-adds + copies +
    reciprocal + normalize, GpSimd = masks + DMA descriptor-gen.
"""

import numpy as np

B, C, S, H, D = 8, 768, 384, 12, 64
NC_CHUNKS = C // 128  # 6

_STATE = {}


# --------------------------------------------------------------------------
# Workaround: this walrus build rejects the multi-wait InstDrain that
# TileContext emits at exit ("Too many sync wait commands"). Split the
# drain's sem waits onto standalone sync-engine wait instructions.
def _patch_walrus_ldw_opt():
    """Enable walrus's load-weight pipelining (ldw-opt): overlaps each
    matmul's LDWEIGHTS with the previous matmul's execution."""
    import os
    import concourse.bass_utils as bu

    if os.environ.get("KERNEL_LDW_OPT") != "1":
        return
    if getattr(bu, "_ldw_opt_patch", False):
        return
    orig = bu.run_command

    def patched(argv, **kwargs):
        argv = [
            a.replace("--enable-ldw-opt=false", "--enable-ldw-opt=true")
            if isinstance(a, str)
            else a
            for a in argv
        ]
        return orig(argv, **kwargs)

    bu.run_command = patched
    bu._ldw_opt_patch = True


def _patch_tile_drain():
    import concourse.tile as tile_mod
    from concourse.vector_clock import ScopedClock
    from bass_rust import SyncInfo

    if getattr(tile_mod.TileContext, "_drain_split_patch", False):
        return

    def _drain_and_barrier_split(self, tick_clock, wait_clock):
        nc = self.nc
        assert self.sems is not None
        handles = {}
        for h in self.sems.allocated().values():
            handles[h.num] = h
            handles[h.name] = h

        probe = nc.sync.nop()
        wait_clock.add_sem_waits(
            probe.ins, ScopedClock({None: tick_clock.global_clock})
        )
        waits = list(probe.ins.sync_info.on_wait)
        probe.ins.sync_info = SyncInfo(on_wait=[], on_update=[])
        for w in waits:
            h = handles.get(w.id) or handles.get(w.ant_name)
            if h is not None:
                nc.sync.wait_ge(h, w.wait_value)
            else:
                n2 = nc.sync.nop()
                n2.ins.sync_info = SyncInfo(on_wait=[w], on_update=[])

        drain_inst = nc.sync.drain()
        wait_clock.add_sem_waits(
            drain_inst.ins, ScopedClock({None: tick_clock.global_clock})
        )
        if list(drain_inst.ins.sync_info.on_wait):
            drain_inst.ins.sync_info = SyncInfo(on_wait=[], on_update=[])

        nc.all_engine_barrier()
        popped = nc._tile_sem_poison_stack.pop()
        assert popped is self._sem_poison
        nc.clear_and_free_semaphores(list(self.sems.allocated().values()))
        nc.all_engine_barrier()

        # This walrus codegen supports at most ONE sem wait per
        # instruction. Move extra waits onto same-engine nop carriers
        # inserted just before the instruction (engine queues execute in
        # order, so the semantics are identical).
        import concourse.mybir as mybir

        k = 0
        for f in nc.m.functions:
            for bb in f.blocks:
                new_insts = []
                for inst in bb.instructions:
                    si = inst.sync_info
                    waits = list(si.on_wait) if si else []
                    if len(waits) > 1:
                        for w in waits[:-1]:
                            nop = mybir.InstNoOp(name=f"I-wsplit-{k}")
                            k += 1
                            nop.engine = inst.engine
                            nop.sync_info = SyncInfo(on_wait=[w], on_update=[])
                            nc.register_instruction(nop)
                            new_insts.append(nop)
                        inst.sync_info = SyncInfo(
                            on_wait=[waits[-1]], on_update=list(si.on_update)
                        )
                    new_insts.append(inst)
                bb.instructions = new_insts

    tile_mod.TileContext._drain_and_barrier = _drain_and_barrier_split
    tile_mod.TileContext._drain_split_patch = True


# --------------------------------------------------------------------------
def _build_nc():
    import concourse.bass as bass
    import concourse.mybir as mybir
    import concourse.tile as tile

    _patch_tile_drain()
    _patch_walrus_ldw_opt()

    f32 = mybir.dt.float32
    f32r = mybir.dt.float32r
    bf16 = mybir.dt.bfloat16
    Ident = mybir.ActivationFunctionType.Identity
    Copy = mybir.ActivationFunctionType.Copy
    Exp = mybir.ActivationFunctionType.Exp

    nc = bass.Bass()
    # All tensors arrive host-packed in their exact SBUF layouts so every
    # DMA moves one contiguous multi-KB run per partition (DMA throughput
    # is descriptor-rate bound; descriptors cover one partition row each).
    # t1 = [x | wq chunk0 | wk chunk0] — one DMA gates the first
    # projections (in-queue completion order across separate DMAs is
    # unreliable). t2 = remaining wq/wk slices grouped per out-chunk.
    t1_d = nc.dram_tensor("t1", [128, 3840], bf16, kind="ExternalInput")
    t2_d = [
        nc.dram_tensor(f"t2{i}", [128, 1, 2, NC_CHUNKS, 128], bf16, kind="ExternalInput")
        for i in range(5)
    ]
    wv_d = nc.dram_tensor("wvt", [128, 2, NC_CHUNKS, 384], bf16, kind="ExternalInput")
    wo_d = nc.dram_tensor("wot", [128, NC_CHUNKS, C], bf16, kind="ExternalInput")
    # packed constants [128, 146] f32:
    #   cols 0:6 bq (col=chunk), 6:12 bk, 12:18 obias (= Wo @ bv'),
    #   cols 18:146 = [128, 256] bf16 = 0/1 lower-triangle (k>=q) twice
    cst_d = nc.dram_tensor("cst", [128, 146], f32, kind="ExternalInput")
    sel_d = nc.dram_tensor("sel", [2, 128], f32, kind="ExternalInput")
    y_d = nc.dram_tensor("y", [128, NC_CHUNKS, S], bf16, kind="ExternalOutput")

    with tile.TileContext(nc) as tc:
        with (
            tc.tile_pool(name="persist", bufs=1) as persist,
            tc.tile_pool(name="epool", bufs=9) as epool,
            tc.tile_pool(name="psA", bufs=2, space="PSUM") as psA,
            tc.tile_pool(name="psS", bufs=4, space="PSUM") as psS,
            tc.tile_pool(name="psV", bufs=2, space="PSUM") as psV,
        ):
            # ---- input loads ------------------------------------------
            # DMA throughput is descriptor-rate bound (one descriptor per
            # partition row), so every tensor arrives host-packed in its
            # exact SBUF layout: each DMA moves one contiguous 1.5-9KB run
            # per partition. Only the DMAs the first projection needs are
            # emitted before it: the tile framework lowers multi-dep waits
            # to one conservative per-queue semaphore value, so any DMA
            # emitted earlier on the same queue would false-serialize it.
            t1 = persist.tile([128, 3840], bf16, tag="t1", name="t1")
            t2 = persist.tile(
                [128, 5, 2, NC_CHUNKS, 128], bf16, tag="t2", name="t2"
            )
            wv_sb = persist.tile([128, 2, NC_CHUNKS, 384], bf16, tag="wv", name="wv")
            wo_sb = persist.tile([128, NC_CHUNKS, C], bf16, tag="wo", name="wo")
            cst = persist.tile([128, 146], f32, tag="cst", name="cst")

            xt = t1[:, 0:2304].rearrange("p (cc s) -> p cc s", s=S)
            wqk0 = t1[:, 2304:3840].rearrange("p (w cc c) -> p w cc c", w=2, c=128)

            def wslice(w, oc, cc):
                # w: 0 = wq, 1 = wk; chunk 0 lives in t1, the rest in t2
                if oc == 0:
                    return wqk0[:, w, cc, :]
                return t2[:, oc - 1, w, cc, :]

            # DMA rings on one queue run CONCURRENTLY (not FIFO), so a
            # later dma_start steals bandwidth from an earlier one. Only
            # t1 + tiny constants issue up front; each later load is
            # released from an engine-queue position that already depends
            # on prior data (staged below in the schedule).
            nc.scalar.dma_start(t1[:], t1_d[:, :])
            nc.gpsimd.dma_start(cst[:], cst_d[:, :])

            def stage_load(stage):
                if stage < 2:
                    # wq/wk chunks 1-2 right behind t1 on the scalar queue
                    nc.scalar.dma_start(
                        t2[:, stage : stage + 1], t2_d[stage][:, :, :, :, :]
                    )
                elif stage == 2:
                    nc.scalar.dma_start(wv_sb[:], wv_d[:, :, :, :])
                elif stage == 3:
                    for i in (2, 3, 4):
                        nc.gpsimd.dma_start(
                            t2[:, i : i + 1], t2_d[i][:, :, :, :, :]
                        )
                elif stage == 4:
                    nc.gpsimd.dma_start(wo_sb[:], wo_d[:, :, :])

            # [128, 2, 128] view of the doubled 0/1 triangle
            tri2 = cst[:, 18:146].bitcast(bf16).rearrange("p (a q) -> p a q", q=128)

            # ---- on-chip constants -----------------------------------
            # vt[sq][k_local, h, 0:64] = V'[c', s]^T ; col 64 = 1.0 (fused
            # softmax-denominator column). sel2 = 0/1 selector for the K=2
            # normalize broadcast (row p lights up partition half p).
            vt = []
            for sq in range(3):
                t = persist.tile([128, H, D + 1], bf16, tag=f"vt{sq}", name=f"vt{sq}")
                nc.gpsimd.memset(t[:, :, D : D + 1], 1.0)
                vt.append(t)
            sel2 = persist.tile([2, 128], f32r, tag="sel2", name="sel2")
            nc.gpsimd.dma_start(sel2[:], sel_d[:, :].bitcast(f32r))

            # ---- persistent working tiles ----------------------------
            q_sb = [
                persist.tile([128, S], bf16, tag=f"q{oc}", name=f"q{oc}")
                for oc in range(NC_CHUNKS)
            ]
            k_sb = [
                persist.tile([128, S], bf16, tag=f"k{oc}", name=f"k{oc}")
                for oc in range(NC_CHUNKS)
            ]
            attn_sb = [
                persist.tile([128, S], bf16, tag=f"at{oc}", name=f"at{oc}")
                for oc in range(NC_CHUNKS)
            ]
            # unnormalized attn [rows 0:64] with the fused softmax
            # denominator as row 64, one column block per head (ordered
            # parity-major within each normalize group so the sums-gather
            # DMA reads one contiguous strip of partition 64)
            AU_g = [
                persist.tile([D + 1, 8, S], f32, tag="au0", name="au0"),
                persist.tile([D + 1, 4, S], f32, tag="au1", name="au1"),
            ]
            sums_g = [
                persist.tile([24, 128], f32, tag="sm0", name="sm0"),
                persist.tile([12, 128], f32, tag="sm1", name="sm1"),
            ]
            rinv_g = [
                persist.tile([24, 128], f32, tag="ri0", name="ri0"),
                persist.tile([12, 128], f32, tag="ri1", name="ri1"),
            ]
            rr_g = [
                persist.tile([2, 4, S], f32r, tag="rr0", name="rr0"),
                persist.tile([2, 2, S], f32r, tag="rr1", name="rr1"),
            ]
            ot = persist.tile([128, NC_CHUNKS, S], bf16, tag="ot", name="ot")

            # ---- stage helpers ---------------------------------------
            def qkproj(oc, w, bcol, out_sb):
                ps = psA.tile([128, S], f32, tag="proj", name="proj")
                for cc in range(NC_CHUNKS):
                    nc.tensor.matmul(
                        ps[:],
                        wslice(w, oc, cc),
                        xt[:, cc, :],
                        start=(cc == 0),
                        stop=(cc == NC_CHUNKS - 1),
                    )
                nc.vector.tensor_scalar_add(out_sb[:], ps[:], cst[:, bcol : bcol + 1])

            def vproj(sq, half):
                ps = psA.tile([128, S], f32, tag="proj", name="proj")
                for cc in range(NC_CHUNKS):
                    nc.tensor.matmul(
                        ps[:],
                        xt[:, cc, sq * 128 : (sq + 1) * 128],
                        wv_sb[:, half, cc, :],
                        start=(cc == 0),
                        stop=(cc == NC_CHUNKS - 1),
                    )
                dst = vt[sq][:, half * 6 : (half + 1) * 6, 0:D]
                src = ps[:].rearrange("p (h d) -> p h d", d=D)
                if half == 0:
                    nc.vector.tensor_copy(dst, src)
                else:
                    nc.scalar.activation(dst, src, Copy)

            def scores(h):
                # psum tile A: [kc0 | kc1] (cols 0:128 = q0:128 over keys
                # 0:128; cols 128:384 = q0:256 over keys 128:256), tile B:
                # kc2 q0:384. exp straight from PSUM; 0/1 triangle applied
                # after on the three diagonal sub-blocks (gpsimd, SBUF).
                oc, prow = h // 2, (h % 2) * D
                Qh = q_sb[oc][prow : prow + D, :]
                Kh = k_sb[oc][prow : prow + D, :]
                psa = psS.tile([128, S], f32, tag="sc", name="sc")
                nc.tensor.matmul(
                    psa[:, 0:128], Kh[:, 0:128], Qh[:, 0:128],
                    start=True, stop=True, skip_group_check=True,
                )
                nc.tensor.matmul(
                    psa[:, 128:384], Kh[:, 128:256], Qh[:, 0:256],
                    start=True, stop=True, skip_group_check=True,
                )
                psb = psS.tile([128, S], f32, tag="sc", name="sc")
                nc.tensor.matmul(
                    psb[:], Kh[:, 256:384], Qh[:, 0:384], start=True, stop=True,
                )
                # eA is 512 wide so its two diagonal sub-blocks (cols 0:128
                # and 256:384) form one uniform-stride [128,2,128] AP for a
                # single masked multiply. Late heads mask on DVE (fast bf16
                # SBUF path) to shorten the tail chase; earlier heads on the
                # otherwise-idle GpSimd.
                eA = epool.tile([128, 512], bf16, tag="eA", name="eA")
                nc.scalar.activation(eA[:, 0:S], psa[:], Exp)
                eB = epool.tile([128, S], bf16, tag="eB", name="eB")
                nc.scalar.activation(eB[:], psb[:], Exp)
                eng = nc.vector if h >= 10 else nc.gpsimd
                diag2 = eA[:].rearrange("p (a q) -> p a q", q=256)[:, :, 0:128]
                eng.tensor_mul(diag2, diag2, tri2)
                eng.tensor_mul(eB[:, 256:384], eB[:, 256:384], tri2[:, 0, :])
                return eA, eB

            def av(h, eA, eB):
                # accumulate widest first so every element's first write
                # carries the start flag
                ps_av = psV.tile([D + 1, S], f32, tag="av", name="av")
                nc.tensor.matmul(
                    ps_av[:, 0:384], vt[2][:, h, :], eB[:, 0:384],
                    start=True, stop=False, skip_group_check=True,
                )
                nc.tensor.matmul(
                    ps_av[:, 0:256], vt[1][:, h, :], eA[:, 128:384],
                    start=False, stop=False, skip_group_check=True,
                )
                nc.tensor.matmul(
                    ps_av[:, 0:128], vt[0][:, h, :], eA[:, 0:128],
                    start=False, stop=True, skip_group_check=True,
                )
                g = 0 if h < 8 else 1
                su_idx = (h % 2) * 4 + h // 2 if h < 8 else (h % 2) * 2 + (h - 8) // 2
                dst = AU_g[g][0 : D + 1, su_idx, :]
                if h >= 4 and h % 2 == 1:
                    # split the end-phase copies across Scalar and DVE
                    nc.scalar.activation(dst, ps_av[:, :], Copy)
                else:
                    nc.vector.tensor_copy(dst, ps_av[:, :])

            def norm_sums_dma(g, oc0, n_oc):
                # sums -> [3n,128] parity-major rows (reciprocal is
                # free-dim bound, so spread over partitions)
                n_h = 2 * n_oc
                nc.sync.dma_start(sums_g[g][:], AU_g[g][D : D + 1, 0:n_h, :])

            def norm_recip(g):
                # reciprocal, then one repack DMA back to two partitions
                # (row = head parity)
                nc.vector.reciprocal(rinv_g[g][:], sums_g[g][:])
                nc.sync.dma_start(rr_g[g][:], rinv_g[g][:].bitcast(f32r))

            def norm_apply(g, oc0, j):
                # per chunk: one K=2 broadcast matmul lights each partition
                # half with its head's 1/sum row, two half-chunk multiplies
                oc = oc0 + j
                ps_r = psS.tile([128, S], f32, tag="sc", name="sc")
                nc.tensor.matmul(
                    ps_r[:], sel2[:], rr_g[g][:, j, :], start=True, stop=True,
                )
                for par in range(2):
                    h = 2 * oc + par
                    su_idx = (
                        (h % 2) * 4 + h // 2 if h < 8 else (h % 2) * 2 + (h - 8) // 2
                    )
                    nc.vector.tensor_mul(
                        attn_sb[oc][par * D : (par + 1) * D, :],
                        AU_g[g][0:D, su_idx, :],
                        ps_r[par * D : (par + 1) * D, :],
                    )

            o_ps = {}

            def oproj(oc, ccs, start, stop):
                if oc in o_ps:
                    ps = o_ps[oc]
                else:
                    ps = o_ps[oc] = psA.tile([128, S], f32, tag="proj", name="proj")
                for i, cc in enumerate(ccs):
                    nc.tensor.matmul(
                        ps[:],
                        wo_sb[:, cc, oc * 128 : (oc + 1) * 128],
                        attn_sb[cc],
                        start=(start and i == 0),
                        stop=(stop and i == len(ccs) - 1),
                        skip_group_check=True,
                    )
                if stop:
                    del o_ps[oc]
                    nc.scalar.activation(
                        ot[:, oc, :], ps[:], Ident, bias=cst[:, 12 + oc : 13 + oc]
                    )
                    if oc % 2 == 1:
                        nc.sync.dma_start(
                            y_d[:, oc - 1 : oc + 1, :], ot[:, oc - 1 : oc + 1, :]
                        )

            # ---- schedule --------------------------------------------
            # oc0-2 projections+scores run while wv loads; V-proj next;
            # then attn@V of chunk oc-3 leads each chunk's projections so
            # the Scalar exp latency hides behind independent PE work.
            # Normalization is grouped (heads 0-7, 8-11); o_proj chains
            # for oc0/oc1 open early to fill the group-1 reciprocal stall.
            e_tiles = {}
            for oc in (0, 1, 2, 3):
                qkproj(oc, 0, oc, q_sb[oc])
                qkproj(oc, 1, 6 + oc, k_sb[oc])
                for h in (2 * oc, 2 * oc + 1):
                    e_tiles[h] = scores(h)
                    if oc == 0:
                        stage_load(h)  # stages 0, 1
                if oc == 1:
                    stage_load(2)  # wv
                    stage_load(3)  # wq/wk chunks 3-5
                if oc == 2:
                    stage_load(4)  # wo
            for sq in range(3):
                for half in range(2):
                    vproj(sq, half)
            for oc in (4, 5):
                for h in (2 * oc - 8, 2 * oc - 7):
                    av(h, *e_tiles.pop(h))
                qkproj(oc, 0, oc, q_sb[oc])
                qkproj(oc, 1, 6 + oc, k_sb[oc])
                for h in (2 * oc, 2 * oc + 1):
                    e_tiles[h] = scores(h)
            for h in (4, 5, 6, 7):
                av(h, *e_tiles.pop(h))
            norm_sums_dma(0, 0, 4)
            av(8, *e_tiles.pop(8))
            av(9, *e_tiles.pop(9))
            av(10, *e_tiles.pop(10))
            av(11, *e_tiles.pop(11))
            norm_sums_dma(1, 4, 2)
            norm_recip(0)
            for j in range(4):
                norm_apply(0, 0, j)
            oproj(0, (0, 1, 2, 3), start=True, stop=False)
            oproj(1, (0, 1, 2, 3), start=True, stop=False)
            norm_recip(1)
            norm_apply(1, 4, 0)
            norm_apply(1, 4, 1)
            oproj(0, (4, 5), start=False, stop=True)
            oproj(1, (4, 5), start=False, stop=True)
            for oc in (2, 3, 4, 5):
                oproj(oc, (0, 1, 2, 3, 4, 5), start=True, stop=True)

    return nc


def _get_nc():
    if "nc" not in _STATE:
        _STATE["nc"] = _build_nc()
    return _STATE["nc"]


# --------------------------------------------------------------------------
def _prep_maps(inputs):
    import ml_dtypes

    bf16 = ml_dtypes.bfloat16

    hs = np.asarray(inputs["hidden_states"], dtype=np.float32)
    Wq = np.asarray(inputs["Wq"], dtype=np.float32)
    bq = np.asarray(inputs["bq"], dtype=np.float32)
    Wk = np.asarray(inputs["Wk"], dtype=np.float32)
    bk = np.asarray(inputs["bk"], dtype=np.float32)
    Wv = np.asarray(inputs["Wv"], dtype=np.float32)
    bv = np.asarray(inputs["bv"], dtype=np.float32)
    Wo = np.asarray(inputs["Wo"], dtype=np.float32)

    # head-major channel permutation: c' = h*64 + d  <-  c = d*12 + h
    idx = (np.arange(H)[:, None] + np.arange(D)[None, :] * H).reshape(C)
    scale = float(D) ** -0.5

    wqt = np.ascontiguousarray((scale * Wq[idx, :]).T).astype(bf16)
    wkt = np.ascontiguousarray(Wk[idx, :].T).astype(bf16)
    wvt = np.ascontiguousarray(Wv[idx, :].T).astype(bf16)
    wot = np.ascontiguousarray(Wo.T).astype(bf16)

    # packed constants [128, 146] f32
    cstf = np.zeros((128, 146), dtype=np.float32)
    cstf[:, 0:6] = (scale * bq[idx]).reshape(NC_CHUNKS, 128).T
    cstf[:, 6:12] = bk[idx].reshape(NC_CHUNKS, 128).T
    # V-bias folded through attention (softmax rows sum to 1):
    # out += Wo @ bv'
    cstf[:, 12:18] = (Wo @ bv[idx]).reshape(NC_CHUNKS, 128).T
    # 0/1 triangle: allowed keys are k >= q -> tri[k, q] = 1 iff k >= q
    tri = (
        np.tril(np.ones((128, 128), dtype=np.float32))
        .astype(bf16)
    )
    cstf[:, 18:146] = np.tile(tri, (1, 2)).view(np.float32)

    sel = np.zeros((2, 128), dtype=np.float32)
    sel[0, 0:64] = 1.0
    sel[1, 64:128] = 1.0

    # pack [c_in, c_out] weights into their SBUF layouts (see _build_nc)
    nch = NC_CHUNKS
    wqp = np.ascontiguousarray(
        wqt.reshape(nch, 128, nch, 128).transpose(1, 2, 0, 3)
    )  # [p, out_chunk, in_chunk, col]
    wkp = np.ascontiguousarray(wkt.reshape(nch, 128, nch, 128).transpose(1, 2, 0, 3))
    wvp = np.ascontiguousarray(
        wvt.reshape(nch, 128, 2, 384).transpose(1, 2, 0, 3)
    )  # [p, half, in_chunk, col]
    wop = np.ascontiguousarray(wot.reshape(nch, 128, C).transpose(1, 0, 2))

    t2 = np.stack([wqp[:, 1:6], wkp[:, 1:6]], axis=2)  # [128, 5, 2, 6, 128]
    w0 = np.concatenate(
        [wqp[:, 0].reshape(128, 768), wkp[:, 0].reshape(128, 768)], axis=1
    )
    shared = {"wvt": wvp, "wot": wop, "cst": cstf, "sel": sel}
    for i in range(5):
        shared[f"t2{i}"] = np.ascontiguousarray(t2[:, i : i + 1])
    maps = []
    for b in range(B):
        xb = hs[b, :, 0, :].astype(bf16)
        xp = xb.reshape(nch, 128, S).transpose(1, 0, 2).reshape(128, nch * S)
        t1 = np.ascontiguousarray(np.concatenate([xp, w0], axis=1))
        maps.append({"t1": t1, **shared})
    return maps


def _run(inputs, trace=False, **kwargs):
    from concourse.bass_utils import run_bass_kernel_spmd

    nc = _get_nc()
    in_maps = _prep_maps(inputs)
    res = run_bass_kernel_spmd(
        nc, in_maps, core_ids=list(range(B)), trace=trace, **kwargs
    )
    out = np.stack(
        [
            np.asarray(res.results[b]["y"])
            .astype(np.float32)
            .transpose(1, 0, 2)  # [p, cc, s] -> [cc, p, s]
            .reshape(C, S)
            for b in range(B)
        ],
        axis=0,
    )
    return out.reshape(B, C, 1, S), res


def kernel(**inputs):
    out, _ = _run(inputs, trace=False)
    return out
